# revision 3
# baseline (speedup 1.0000x reference)
"""Trainium2 Bass kernel for MC-sampled cross-entropy-with-variance loss.

loss = mean_{s,b,h,w}[ logsumexp_c(mean + std*eps[s]) - logit[label] ]

Device computes the big term sum_{s,b,pix} ln(sum_c exp(mean + std*eps));
the label term (a 19x-smaller gather, sum_s logit[label]) is folded in on
the host alongside the other input prep (std=exp(0.5*log_var), bf16 casts).

Per-core layout: the core's pixel slab [S,B,C,32768] is viewed as a flat
row axis [S*1216, 2048] (row = (s,b,c,chunk), 2048 pixels each). Each
1216-row s-block is padded to 1280 = 10x128 so all engines see full
[128, 10240] supertiles (2 per s-block):
- DMA: one contiguous 2.62MB transfer per supertile, all on the gpsimd
  SWDGE queue (sync hosts the framework's semaphore traffic); sd/mean
  load as two halves each so the first mul doesn't wait for all 10MB
- DVE: t = eps*sd, x = t + mean as two [128,10240] bf16 tensor_tensor
  ops (2x mode), ping-ponging between the eps and t buffers; eps pool
  is triple-buffered so the DMA for supertile k+3 overlaps compute
- ACT: e1 = exp(x) in one [128,10240] op
- PE:  class-sums via 0/1 selector matmuls, 4 x 512-col per tile, into a
  [65, 2048] f32 PSUM block per s-block (psum row = b*16+chunk, row 64 =
  trash for the 64 pad rows); accumulation start/stop spans the 10 tiles
- ACT: ln on the PSUM block in place with accum_out -> [65,1]; the ln
  for s-block s is issued after the first exp of s-block s+1 so it never
  stalls on the matmul tail. Compile is steered to the combined
  natural_log_exp_and_others ACT table set (one table load total).

sd/mean (s-invariant) stay resident in SBUF as [128, 10240] bf16 half
tiles (partition p holds rows {128t+p}); supertile h of every s-block
aligns exactly with sd/mean half h since 640 = 5*128.
"""

import numpy as np
import ml_dtypes

import concourse.bass as bass
import concourse.bacc as bacc
import concourse.mybir as mybir
from concourse import tile
from concourse.bass_interp import get_hw_module
from concourse.bass_utils import run_bass_kernel_spmd
from concourse.mybir import ActivationFunctionType as Act

# ---------------------------------------------------------------- sizes
S, B, C, H, W = 10, 4, 19, 512, 512
HW = H * W
NCORES = 8
SLAB = HW // NCORES          # pixels per (core, b) = 32768
F = 2048                     # pixels per chunk row
CH = SLAB // F               # 16 chunks per (b, c)
QB = B * C * CH              # 1216 real rows per s-block
QP = 1280                    # padded rows per s-block (10 x 128)
NT = QP // 128               # 10 tiles per s-block
ST = 5                       # tiles per supertile
STF = ST * F                 # 10240 free elems per supertile
NROW = 65                    # psum rows: 64 real (b,chunk) + 1 trash
F32 = mybir.dt.float32
BF16 = mybir.dt.bfloat16


def _compile_combined_act_set(nc):
    """Steer insert_act_table_loads to the one table set that holds both
    Exp and Ln, so the kernel issues a single ACT_TABLE_LOAD instead of
    toggling exp<->ln sets every s-block."""
    orig = bacc.get_activation_tables
    combined = "natural_log_exp_and_others"

    def patched(arch):
        out = {}
        for name, fns in orig(arch).items():
            if name != combined:
                fns = fns - {Act.Exp, Act.Ln}
            out[name] = fns
        return out

    bacc.get_activation_tables = patched
    try:
        nc.compile()
    finally:
        bacc.get_activation_tables = orig


def build_program():
    nc = bacc.Bacc("TRN2", target_bir_lowering=False, debug=False,
                   num_devices=NCORES)

    eps_h = nc.dram_tensor("eps_s", [S * QB + 64, F], BF16,
                           kind="ExternalInput")
    sd_h = nc.dram_tensor("sd_s", [QP, F], BF16, kind="ExternalInput")
    mean_h = nc.dram_tensor("mean_s", [QP, F], BF16, kind="ExternalInput")
    sel_h = nc.dram_tensor("sel_s", [128, NT * NROW], BF16,
                           kind="ExternalInput")
    lse_h = nc.dram_tensor("lse_out", [64, 1], F32, kind="ExternalOutput")

    def flat_ap(handle, row0, ntiles):
        """[128, ntiles*F] view: partition p, free (t, col) = row 128t+p."""
        return bass.AP(tensor=handle, offset=row0 * F,
                       ap=[[F, 128], [128 * F, ntiles], [1, F]])

    with tile.TileContext(nc) as tc:
        with (
            tc.tile_pool(name="consts", bufs=1) as consts,
            tc.tile_pool(name="epsp", bufs=3) as eps_pool,
            tc.tile_pool(name="tp", bufs=2) as t_pool,
            tc.tile_pool(name="post", bufs=2) as post,
            tc.tile_pool(name="accp", bufs=1) as acc_pool,
            tc.tile_pool(name="psum", bufs=2, space="PSUM") as psum_pool,
        ):
            sel_t = consts.tile([128, NT * NROW], BF16, tag="sel")
            nc.sync.dma_start(out=sel_t, in_=sel_h.ap())
            sd_sb = []
            mean_sb = []
            for h in range(2):
                sd_half = consts.tile([128, STF], BF16, tag=f"sd{h}")
                nc.sync.dma_start(out=sd_half,
                                  in_=flat_ap(sd_h, 640 * h, ST))
                mean_half = consts.tile([128, STF], BF16, tag=f"mean{h}")
                nc.scalar.dma_start(out=mean_half,
                                    in_=flat_ap(mean_h, 640 * h, ST))
                sd_sb.append(sd_half)
                mean_sb.append(mean_half)

            acc = acc_pool.tile([64, 1], F32)
            nc.vector.memset(acc, 0.0)

            pending = None   # (psum tile, s) awaiting its deferred ln
            psum_prev = None
            for s in range(S):
                psum_t = psum_pool.tile([NROW, 4 * 512], F32, tag="ps")
                for h in range(2):
                    eps_t = eps_pool.tile([128, STF], BF16, tag="eps")
                    nc.gpsimd.dma_start(
                        out=eps_t,
                        in_=flat_ap(eps_h, QB * s + 640 * h, ST))
                    t_t = t_pool.tile([128, STF], BF16, tag="t")
                    nc.vector.tensor_mul(t_t, eps_t, sd_sb[h])
                    nc.vector.tensor_add(eps_t, t_t, mean_sb[h])
                    nc.scalar.activation(t_t, eps_t, Act.Exp)
                    for j in range(ST):
                        tt = ST * h + j
                        ssl = slice(tt * NROW, (tt + 1) * NROW)
                        for m in range(4):
                            msl = slice(j * F + m * 512,
                                        j * F + (m + 1) * 512)
                            nc.tensor.matmul(
                                psum_t[:, m * 512:(m + 1) * 512],
                                sel_t[:, ssl], t_t[:, msl],
                                start=(tt == 0), stop=(tt == NT - 1))
                    if h == 0 and psum_prev is not None:
                        # deferred ln for the previous s-block: its matmul
                        # tail completed during this supertile's exp
                        lse_p = post.tile([NROW, 1], F32, tag="lsep")
                        nc.scalar.activation(psum_prev, psum_prev, Act.Ln,
                                             accum_out=lse_p)
                        nc.vector.tensor_add(acc, acc, lse_p[0:64])
                psum_prev = psum_t

            lse_p = post.tile([NROW, 1], F32, tag="lsep")
            nc.scalar.activation(psum_prev, psum_prev, Act.Ln,
                                 accum_out=lse_p)
            nc.vector.tensor_add(acc, acc, lse_p[0:64])
            nc.sync.dma_start(out=lse_h.ap(), in_=acc)

    _compile_combined_act_set(nc)
    nc.m = get_hw_module(nc.m)
    return nc


def _selectors():
    """[128, NT*NROW] 0/1 routing: tile tt row i -> psum row b*16+chunk."""
    bf = ml_dtypes.bfloat16
    sel = np.zeros((128, NT, NROW), dtype=bf)
    for tt in range(NT):
        for i in range(128):
            q = 128 * tt + i
            if q < QB:
                b, rem = divmod(q, C * CH)
                c, chunk = divmod(rem, CH)
                sel[i, tt, b * CH + chunk] = 1.0
            else:
                sel[i, tt, 64] = 1.0
    return sel.reshape(128, NT * NROW)


def kernel(mean, log_var, label, eps, _trace=False):
    bf = ml_dtypes.bfloat16
    mean_r = np.asarray(mean, dtype=np.float32).reshape(B, C, HW)
    log_var_r = np.asarray(log_var, dtype=np.float32).reshape(B, C, HW)
    lab = np.asarray(label).reshape(B, HW)
    eps_r = np.asarray(eps, dtype=np.float32).reshape(S, B, C, HW)

    std = np.exp(0.5 * log_var_r)
    sd_bf = std.astype(bf)
    mean_bf = mean_r.astype(bf)
    eps_bf = eps_r.astype(bf)

    sel = _selectors()
    pad_sd = np.zeros((64, F), dtype=bf)
    pad_eps = np.zeros((64, F), dtype=bf)
    in_maps = []
    for cid in range(NCORES):
        lo, hi = cid * SLAB, (cid + 1) * SLAB
        in_maps.append({
            "eps_s": np.concatenate(
                [np.ascontiguousarray(
                    eps_bf[:, :, :, lo:hi]).reshape(S * QB, F), pad_eps]),
            "sd_s": np.concatenate(
                [np.ascontiguousarray(
                    sd_bf[:, :, lo:hi]).reshape(QB, F), pad_sd]),
            "mean_s": np.concatenate(
                [np.ascontiguousarray(
                    mean_bf[:, :, lo:hi]).reshape(QB, F), pad_sd]),
            "sel_s": sel,
        })

    nc = build_program()
    res = run_bass_kernel_spmd(
        nc, in_maps, core_ids=list(range(NCORES)), trace=_trace
    )
    global last_results
    last_results = res

    lse_total = np.float64(0.0)
    for cid in range(NCORES):
        lse_total += res.results[cid]["lse_out"].astype(np.float64).sum()

    # label term on host: sum_{s,b,pix} (mean[lab] + std[lab]*eps[s,lab])
    meanl = np.take_along_axis(mean_r, lab[:, None, :], axis=1)[:, 0, :]
    stdl = np.take_along_axis(std, lab[:, None, :], axis=1)[:, 0, :]
    epsl = np.take_along_axis(
        eps_r, lab[None, :, None, :], axis=2)[:, :, 0, :]
    label_total = (S * meanl.astype(np.float64).sum()
                   + (stdl * epsl.sum(axis=0)).astype(np.float64).sum())

    loss = (lse_total - label_total) / float(S * B * HW)
    return np.float32(loss)


# revision 6
# speedup vs baseline: 1.0126x; 1.0126x over previous
"""Trainium2 Bass kernel for MC-sampled cross-entropy-with-variance loss.

loss = mean_{s,b,h,w}[ logsumexp_c(mean + std*eps[s]) - logit[label] ]

Device computes the big term sum_{s,b,pix} ln(sum_c exp(mean + std*eps));
the label term (a 19x-smaller gather, sum_s logit[label]) is folded in on
the host alongside the other input prep (std=exp(0.5*log_var), bf16 casts).

Per-core layout: the core's pixel slab [S,B,C,32768] is viewed as a flat
row axis [S*1216, 2048] (row = (s,b,c,chunk), 2048 pixels each). Each
1216-row s-block is padded to 1280 = 10x128 rows and split into two
[128, 10240] supertiles (SBUF partition p of supertile k holds flat rows
{640k+128j+p}). The host pre-arranges eps/sd/mean in exactly this
partition-major layout, so one supertile DMA is 128 contiguous 20KB
lines (128 descriptors) - a single SWDGE (gpsimd) queue sustains the
pipeline cadence without a second DMA stream contending with the DVE
for SBUF ports (a second stream measurably slows every DVE op ~20%).

Per supertile:
- DVE: t = eps*sd, x = t + mean as two [128,10240] bf16 tensor_tensor
  ops (2x mode), ping-ponging between the eps and t buffers; eps pool
  is quad-buffered so DMAs stay ahead
- ACT: e1 = exp(x) in one op
- PE:  class-sums via 0/1 selector matmuls, 4 x 512-col per tile, into a
  [65, 2048] f32 PSUM block per s-block (psum row = b*16+chunk, row 64 =
  trash for the 64 pad rows); accumulation start/stop spans the 10 tiles
- ACT: ln on the PSUM block in place with accum_out -> [65,1]; the ln
  for s-block s is issued after the first exp of s-block s+1 so it never
  stalls on the matmul tail. Compile is steered to the combined
  natural_log_exp_and_others ACT table set (one table load total).

The first and last supertiles run at single-tile granularity (5 x
[128,2048] units) so the pipeline ramps up in ~10us and drains in ~10us
instead of waiting on whole-supertile dependencies.
"""

import numpy as np
import ml_dtypes

import concourse.bass as bass
import concourse.bacc as bacc
import concourse.mybir as mybir
from concourse import tile
from concourse.bass_interp import get_hw_module
from concourse.bass_utils import run_bass_kernel_spmd
from concourse.mybir import ActivationFunctionType as Act

# ---------------------------------------------------------------- sizes
S, B, C, H, W = 10, 4, 19, 512, 512
HW = H * W
NCORES = 8
SLAB = HW // NCORES          # pixels per (core, b) = 32768
F = 2048                     # pixels per chunk row
CH = SLAB // F               # 16 chunks per (b, c)
QB = B * C * CH              # 1216 real rows per s-block
QP = 1280                    # padded rows per s-block (10 x 128)
NT = QP // 128               # 10 tiles per s-block
ST = 5                       # tiles per supertile
STF = ST * F                 # 10240 free elems per supertile
NK = 2 * S                   # 20 supertiles
NROW = 65                    # psum rows: 64 real (b,chunk) + 1 trash
F32 = mybir.dt.float32
BF16 = mybir.dt.bfloat16


def _compile_combined_act_set(nc):
    """Steer insert_act_table_loads to the one table set that holds both
    Exp and Ln, so the kernel issues a single ACT_TABLE_LOAD instead of
    toggling exp<->ln sets every s-block."""
    orig = bacc.get_activation_tables
    combined = "natural_log_exp_and_others"

    def patched(arch):
        out = {}
        for name, fns in orig(arch).items():
            if name != combined:
                fns = fns - {Act.Exp, Act.Ln}
            out[name] = fns
        return out

    bacc.get_activation_tables = patched
    try:
        nc.compile()
    finally:
        bacc.get_activation_tables = orig


def build_program():
    nc = bacc.Bacc("TRN2", target_bir_lowering=False, debug=False,
                   num_devices=NCORES)

    # partition-major: row k*128+p holds the 10240 free elems of
    # supertile k's partition p
    eps_h = nc.dram_tensor("eps_s", [NK * 128, STF], BF16,
                           kind="ExternalInput")
    sd_h = nc.dram_tensor("sd_s", [2 * 128, STF], BF16,
                          kind="ExternalInput")
    mean_h = nc.dram_tensor("mean_s", [2 * 128, STF], BF16,
                            kind="ExternalInput")
    sel_h = nc.dram_tensor("sel_s", [128, NT * NROW], BF16,
                           kind="ExternalInput")
    lse_h = nc.dram_tensor("lse_out", [64, 1], F32, kind="ExternalOutput")

    def sup_ap(handle, k, j=0, w=ST):
        """[128, w*F] partition-major view of supertile k, tiles j..j+w."""
        return bass.AP(tensor=handle, offset=(k * 128 * ST + j) * F,
                       ap=[[STF, 128], [1, w * F]])

    with tile.TileContext(nc) as tc:
        with (
            tc.tile_pool(name="consts", bufs=1) as consts,
            tc.tile_pool(name="epsp", bufs=4) as eps_pool,
            tc.tile_pool(name="tp", bufs=2) as t_pool,
            tc.tile_pool(name="post", bufs=2) as post,
            tc.tile_pool(name="accp", bufs=1) as acc_pool,
            tc.tile_pool(name="psum", bufs=2, space="PSUM") as psum_pool,
        ):
            sel_t = consts.tile([128, NT * NROW], BF16, tag="sel")
            nc.sync.dma_start(out=sel_t, in_=sel_h.ap())
            # first halves tile-by-tile so mul(tile 0) starts ~10us in
            sd_sb = [consts.tile([128, STF], BF16, tag=f"sd{h}",
                                 name=f"sd{h}") for h in range(2)]
            mean_sb = [consts.tile([128, STF], BF16, tag=f"mean{h}",
                                   name=f"mean{h}") for h in range(2)]
            for j in range(ST):
                fsl = slice(j * F, (j + 1) * F)
                nc.sync.dma_start(out=sd_sb[0][:, fsl],
                                  in_=sup_ap(sd_h, 0, j, 1))
                nc.scalar.dma_start(out=mean_sb[0][:, fsl],
                                    in_=sup_ap(mean_h, 0, j, 1))

            acc = acc_pool.tile([64, 1], F32)
            nc.vector.memset(acc, 0.0)

            psum_prev = None
            for s in range(S):
                psum_t = psum_pool.tile([NROW, 4 * 512], F32, tag="ps")
                for h in range(2):
                    k = 2 * s + h
                    fine = k == 0 or k == NK - 1
                    eps_t = eps_pool.tile([128, STF], BF16, tag="eps")
                    if k == 0:
                        for j in range(ST):
                            nc.gpsimd.dma_start(
                                out=eps_t[:, j * F:(j + 1) * F],
                                in_=sup_ap(eps_h, 0, j, 1))
                        nc.sync.dma_start(out=sd_sb[1],
                                          in_=sup_ap(sd_h, 1))
                        nc.scalar.dma_start(out=mean_sb[1],
                                            in_=sup_ap(mean_h, 1))
                    else:
                        nc.gpsimd.dma_start(out=eps_t,
                                            in_=sup_ap(eps_h, k))
                    t_t = t_pool.tile([128, STF], BF16, tag="t")
                    units = ([(j * F, F) for j in range(ST)]
                             if fine else [(0, STF)])
                    for off, w in units:
                        usl = slice(off, off + w)
                        nc.vector.tensor_mul(t_t[:, usl], eps_t[:, usl],
                                             sd_sb[h][:, usl])
                        nc.vector.tensor_add(eps_t[:, usl], t_t[:, usl],
                                             mean_sb[h][:, usl])
                        nc.scalar.activation(t_t[:, usl], eps_t[:, usl],
                                             Act.Exp)
                        for j in range(off // F, (off + w) // F):
                            tt = ST * h + j
                            ssl = slice(tt * NROW, (tt + 1) * NROW)
                            for m in range(4):
                                msl = slice(j * F + m * 512,
                                            j * F + (m + 1) * 512)
                                nc.tensor.matmul(
                                    psum_t[:, m * 512:(m + 1) * 512],
                                    sel_t[:, ssl], t_t[:, msl],
                                    start=(tt == 0), stop=(tt == NT - 1))
                    if h == 0 and psum_prev is not None:
                        # deferred ln for the previous s-block: its matmul
                        # tail completed during this supertile's exp
                        lse_p = post.tile([NROW, 1], F32, tag="lsep")
                        nc.scalar.activation(psum_prev, psum_prev, Act.Ln,
                                             accum_out=lse_p)
                        nc.vector.tensor_add(acc, acc, lse_p[0:64])
                psum_prev = psum_t

            lse_p = post.tile([NROW, 1], F32, tag="lsep")
            nc.scalar.activation(psum_prev, psum_prev, Act.Ln,
                                 accum_out=lse_p)
            nc.vector.tensor_add(acc, acc, lse_p[0:64])
            nc.sync.dma_start(out=lse_h.ap(), in_=acc)

    _compile_combined_act_set(nc)
    nc.m = get_hw_module(nc.m)
    return nc


def _selectors():
    """[128, NT*NROW] 0/1 routing: tile tt row i -> psum row b*16+chunk."""
    bf = ml_dtypes.bfloat16
    sel = np.zeros((128, NT, NROW), dtype=bf)
    for tt in range(NT):
        for i in range(128):
            q = 128 * tt + i
            if q < QB:
                b, rem = divmod(q, C * CH)
                c, chunk = divmod(rem, CH)
                sel[i, tt, b * CH + chunk] = 1.0
            else:
                sel[i, tt, 64] = 1.0
    return sel.reshape(128, NT * NROW)


def _to_partition_major(rows, nblocks):
    """[nblocks*1216, 2048] real rows -> [nblocks*2*128, 10240] where row
    k*128+p = concat_j rows_padded[block, 640*(k%2)+128*j+p]."""
    nb = nblocks
    padded = np.zeros((nb, QP, F), dtype=rows.dtype)
    padded[:, :QB] = rows.reshape(nb, QB, F)
    # [nb, 2, 5, 128, F] -> [nb, 2, 128, 5, F]
    out = padded.reshape(nb, 2, ST, 128, F).swapaxes(2, 3)
    return np.ascontiguousarray(out).reshape(nb * 2 * 128, ST * F)


def kernel(mean, log_var, label, eps, _trace=False):
    bf = ml_dtypes.bfloat16
    mean_r = np.asarray(mean, dtype=np.float32).reshape(B, C, HW)
    log_var_r = np.asarray(log_var, dtype=np.float32).reshape(B, C, HW)
    lab = np.asarray(label).reshape(B, HW)
    eps_r = np.asarray(eps, dtype=np.float32).reshape(S, B, C, HW)

    std = np.exp(0.5 * log_var_r)
    sd_bf = std.astype(bf)
    mean_bf = mean_r.astype(bf)
    eps_bf = eps_r.astype(bf)

    sel = _selectors()
    in_maps = []
    for cid in range(NCORES):
        lo, hi = cid * SLAB, (cid + 1) * SLAB
        in_maps.append({
            "eps_s": _to_partition_major(
                np.ascontiguousarray(eps_bf[:, :, :, lo:hi]), S),
            "sd_s": _to_partition_major(
                np.ascontiguousarray(sd_bf[:, :, lo:hi]), 1),
            "mean_s": _to_partition_major(
                np.ascontiguousarray(mean_bf[:, :, lo:hi]), 1),
            "sel_s": sel,
        })

    nc = build_program()
    res = run_bass_kernel_spmd(
        nc, in_maps, core_ids=list(range(NCORES)), trace=_trace
    )
    global last_results
    last_results = res

    lse_total = np.float64(0.0)
    for cid in range(NCORES):
        lse_total += res.results[cid]["lse_out"].astype(np.float64).sum()

    # label term on host: sum_{s,b,pix} (mean[lab] + std[lab]*eps[s,lab])
    meanl = np.take_along_axis(mean_r, lab[:, None, :], axis=1)[:, 0, :]
    stdl = np.take_along_axis(std, lab[:, None, :], axis=1)[:, 0, :]
    epsl = np.take_along_axis(
        eps_r, lab[None, :, None, :], axis=2)[:, :, 0, :]
    label_total = (S * meanl.astype(np.float64).sum()
                   + (stdl * epsl.sum(axis=0)).astype(np.float64).sum())

    loss = (lse_total - label_total) / float(S * B * HW)
    return np.float32(loss)


# revision 8
# speedup vs baseline: 1.0710x; 1.0577x over previous
"""Trainium2 Bass kernel for MC-sampled cross-entropy-with-variance loss.

loss = mean_{s,b,h,w}[ logsumexp_c(mean + std*eps[s]) - logit[label] ]

Device computes the big term sum_{s,b,pix} ln(sum_c exp(mean + std*eps));
the label term (a 19x-smaller gather, sum_s logit[label]) is folded in on
the host alongside the other input prep (std=exp(0.5*log_var), bf16 casts).

Per-core layout: the core's pixel slab [S,B,C,32768] is viewed as a flat
row axis [S*1216, 2048] (row = (s,b,c,chunk), 2048 pixels each). Each
1216-row s-block is padded to 1280 = 10x128 rows and split into two
[128, 10240] supertiles (SBUF partition p of supertile k holds flat rows
{640k+128j+p}). The host pre-arranges eps/sd/mean in exactly this
partition-major layout, so one supertile DMA is 128 contiguous 20KB
lines (128 descriptors) - a single SWDGE (gpsimd) queue sustains the
pipeline cadence without a second DMA stream contending with the DVE
for SBUF ports (a second stream measurably slows every DVE op ~20%).

Per supertile:
- DVE: t = eps*sd, x = t + mean as two [128,10240] bf16 tensor_tensor
  ops (2x mode), ping-ponging between the eps and t buffers; eps pool
  is quad-buffered so DMAs stay ahead
- ACT: e1 = exp(x) in one op
- PE:  class-sums via 0/1 selector matmuls, 4 x 512-col per tile, into a
  [65, 2048] f32 PSUM block per s-block (psum row = b*16+chunk, row 64 =
  trash for the 64 pad rows); accumulation start/stop spans the 10 tiles
- ACT: ln on the PSUM block in place with accum_out -> [65,1]; the ln
  for s-block s is issued after the first exp of s-block s+1 so it never
  stalls on the matmul tail. Compile is steered to the combined
  natural_log_exp_and_others ACT table set (one table load total).

The first and last supertiles run at single-tile granularity (5 x
[128,2048] units) so the pipeline ramps up in ~10us and drains in ~10us
instead of waiting on whole-supertile dependencies.
"""

import numpy as np
import ml_dtypes

import concourse.bass as bass
import concourse.bacc as bacc
import concourse.mybir as mybir
from concourse import tile
from concourse.bass_interp import get_hw_module
from concourse.bass_utils import run_bass_kernel_spmd
from concourse.mybir import ActivationFunctionType as Act

# ---------------------------------------------------------------- sizes
S, B, C, H, W = 10, 4, 19, 512, 512
HW = H * W
NCORES = 8
SLAB = HW // NCORES          # pixels per (core, b) = 32768
F = 2048                     # pixels per chunk row
CH = SLAB // F               # 16 chunks per (b, c)
QB = B * C * CH              # 1216 real rows per s-block
QP = 1280                    # padded rows per s-block (10 x 128)
NT = QP // 128               # 10 tiles per s-block
ST = 5                       # tiles per supertile
STF = ST * F                 # 10240 free elems per supertile
NK = 2 * S                   # 20 supertiles
NROW = 65                    # psum rows: 64 real (b,chunk) + 1 trash
F32 = mybir.dt.float32
BF16 = mybir.dt.bfloat16


def _compile_combined_act_set(nc):
    """Steer insert_act_table_loads to the one table set that holds both
    Exp and Ln, so the kernel issues a single ACT_TABLE_LOAD instead of
    toggling exp<->ln sets every s-block."""
    orig = bacc.get_activation_tables
    combined = "natural_log_exp_and_others"

    def patched(arch):
        out = {}
        for name, fns in orig(arch).items():
            if name != combined:
                fns = fns - {Act.Exp, Act.Ln}
            out[name] = fns
        return out

    bacc.get_activation_tables = patched
    try:
        nc.compile()
    finally:
        bacc.get_activation_tables = orig


def build_program():
    nc = bacc.Bacc("TRN2", target_bir_lowering=False, debug=False,
                   num_devices=NCORES)

    # partition-major: row k*128+p holds the 10240 free elems of
    # supertile k's partition p
    eps_h = nc.dram_tensor("eps_s", [NK * 128, STF], BF16,
                           kind="ExternalInput")
    sd_h = nc.dram_tensor("sd_s", [2 * 128, STF], BF16,
                          kind="ExternalInput")
    mean_h = nc.dram_tensor("mean_s", [2 * 128, STF], BF16,
                            kind="ExternalInput")
    sel_h = nc.dram_tensor("sel_s", [128, NT * NROW], BF16,
                           kind="ExternalInput")
    lse_h = nc.dram_tensor("lse_out", [64, 1], F32, kind="ExternalOutput")

    def sup_ap(handle, k, j=0, w=ST):
        """[128, w*F] partition-major view of supertile k, tiles j..j+w."""
        return bass.AP(tensor=handle, offset=(k * 128 * ST + j) * F,
                       ap=[[STF, 128], [1, w * F]])

    with tile.TileContext(nc) as tc:
        with (
            tc.tile_pool(name="consts", bufs=1) as consts,
            tc.tile_pool(name="epsp", bufs=4) as eps_pool,
            tc.tile_pool(name="xp", bufs=2) as x_pool,
            tc.tile_pool(name="post", bufs=2) as post,
            tc.tile_pool(name="accp", bufs=1) as acc_pool,
            tc.tile_pool(name="psum", bufs=2, space="PSUM") as psum_pool,
        ):
            sel_t = consts.tile([128, NT * NROW], BF16, tag="sel")
            nc.sync.dma_start(out=sel_t, in_=sel_h.ap())
            # tile-by-tile so mul(tile 0) starts ~10us in
            sd_sb = [consts.tile([128, STF], BF16, tag=f"sd{h}",
                                 name=f"sd{h}") for h in range(2)]
            mean_sb = [consts.tile([128, STF], BF16, tag=f"mean{h}",
                                   name=f"mean{h}") for h in range(2)]
            for hh in range(2):
                for j in range(ST):
                    fsl = slice(j * F, (j + 1) * F)
                    nc.sync.dma_start(out=sd_sb[hh][:, fsl],
                                      in_=sup_ap(sd_h, hh, j, 1))
                    nc.scalar.dma_start(out=mean_sb[hh][:, fsl],
                                        in_=sup_ap(mean_h, hh, j, 1))

            acc = acc_pool.tile([64, 1], F32)
            nc.vector.memset(acc, 0.0)

            psum_prev = None
            for s in range(S):
                psum_t = psum_pool.tile([NROW, 4 * 512], F32, tag="ps")
                for h in range(2):
                    k = 2 * s + h
                    fine = k == 0 or k == NK - 1
                    eps_t = eps_pool.tile([128, STF], BF16, tag="eps")
                    if k == 0:
                        for j in range(ST):
                            nc.gpsimd.dma_start(
                                out=eps_t[:, j * F:(j + 1) * F],
                                in_=sup_ap(eps_h, 0, j, 1))
                    else:
                        nc.gpsimd.dma_start(out=eps_t,
                                            in_=sup_ap(eps_h, k))
                    x_t = x_pool.tile([128, STF], BF16, tag="x")
                    units = ([(j * F, F) for j in range(ST)]
                             if fine else [(0, STF)])
                    for off, w in units:
                        usl = slice(off, off + w)
                        nc.vector.tensor_mul(eps_t[:, usl], eps_t[:, usl],
                                             sd_sb[h][:, usl])
                        nc.vector.tensor_add(x_t[:, usl], eps_t[:, usl],
                                             mean_sb[h][:, usl])
                        nc.scalar.activation(eps_t[:, usl], x_t[:, usl],
                                             Act.Exp)
                        for j in range(off // F, (off + w) // F):
                            tt = ST * h + j
                            ssl = slice(tt * NROW, (tt + 1) * NROW)
                            for m in range(4):
                                msl = slice(j * F + m * 512,
                                            j * F + (m + 1) * 512)
                                nc.tensor.matmul(
                                    psum_t[:, m * 512:(m + 1) * 512],
                                    sel_t[:, ssl], eps_t[:, msl],
                                    start=(tt == 0), stop=(tt == NT - 1))
                    if h == 0 and psum_prev is not None:
                        # deferred ln for the previous s-block: its matmul
                        # tail completed during this supertile's exp
                        lse_p = post.tile([NROW, 1], F32, tag="lsep")
                        nc.scalar.activation(psum_prev, psum_prev, Act.Ln,
                                             accum_out=lse_p)
                        nc.vector.tensor_add(acc, acc, lse_p[0:64])
                psum_prev = psum_t

            lse_p = post.tile([NROW, 1], F32, tag="lsep")
            nc.scalar.activation(psum_prev, psum_prev, Act.Ln,
                                 accum_out=lse_p)
            nc.vector.tensor_add(acc, acc, lse_p[0:64])
            nc.sync.dma_start(out=lse_h.ap(), in_=acc)

    _compile_combined_act_set(nc)
    nc.m = get_hw_module(nc.m)
    return nc


def _selectors():
    """[128, NT*NROW] 0/1 routing: tile tt row i -> psum row b*16+chunk."""
    bf = ml_dtypes.bfloat16
    sel = np.zeros((128, NT, NROW), dtype=bf)
    for tt in range(NT):
        for i in range(128):
            q = 128 * tt + i
            if q < QB:
                b, rem = divmod(q, C * CH)
                c, chunk = divmod(rem, CH)
                sel[i, tt, b * CH + chunk] = 1.0
            else:
                sel[i, tt, 64] = 1.0
    return sel.reshape(128, NT * NROW)


def _to_partition_major(rows, nblocks):
    """[nblocks*1216, 2048] real rows -> [nblocks*2*128, 10240] where row
    k*128+p = concat_j rows_padded[block, 640*(k%2)+128*j+p]."""
    nb = nblocks
    padded = np.zeros((nb, QP, F), dtype=rows.dtype)
    padded[:, :QB] = rows.reshape(nb, QB, F)
    # [nb, 2, 5, 128, F] -> [nb, 2, 128, 5, F]
    out = padded.reshape(nb, 2, ST, 128, F).swapaxes(2, 3)
    return np.ascontiguousarray(out).reshape(nb * 2 * 128, ST * F)


def kernel(mean, log_var, label, eps, _trace=False):
    bf = ml_dtypes.bfloat16
    mean_r = np.asarray(mean, dtype=np.float32).reshape(B, C, HW)
    log_var_r = np.asarray(log_var, dtype=np.float32).reshape(B, C, HW)
    lab = np.asarray(label).reshape(B, HW)
    eps_r = np.asarray(eps, dtype=np.float32).reshape(S, B, C, HW)

    std = np.exp(0.5 * log_var_r)
    sd_bf = std.astype(bf)
    mean_bf = mean_r.astype(bf)
    eps_bf = eps_r.astype(bf)

    sel = _selectors()
    in_maps = []
    for cid in range(NCORES):
        lo, hi = cid * SLAB, (cid + 1) * SLAB
        in_maps.append({
            "eps_s": _to_partition_major(
                np.ascontiguousarray(eps_bf[:, :, :, lo:hi]), S),
            "sd_s": _to_partition_major(
                np.ascontiguousarray(sd_bf[:, :, lo:hi]), 1),
            "mean_s": _to_partition_major(
                np.ascontiguousarray(mean_bf[:, :, lo:hi]), 1),
            "sel_s": sel,
        })

    nc = build_program()
    res = run_bass_kernel_spmd(
        nc, in_maps, core_ids=list(range(NCORES)), trace=_trace
    )
    global last_results
    last_results = res

    lse_total = np.float64(0.0)
    for cid in range(NCORES):
        lse_total += res.results[cid]["lse_out"].astype(np.float64).sum()

    # label term on host: sum_{s,b,pix} (mean[lab] + std[lab]*eps[s,lab])
    meanl = np.take_along_axis(mean_r, lab[:, None, :], axis=1)[:, 0, :]
    stdl = np.take_along_axis(std, lab[:, None, :], axis=1)[:, 0, :]
    epsl = np.take_along_axis(
        eps_r, lab[None, :, None, :], axis=2)[:, :, 0, :]
    label_total = (S * meanl.astype(np.float64).sum()
                   + (stdl * epsl.sum(axis=0)).astype(np.float64).sum())

    loss = (lse_total - label_total) / float(S * B * HW)
    return np.float32(loss)


# revision 9
# speedup vs baseline: 1.1604x; 1.0834x over previous
"""Trainium2 Bass kernel for MC-sampled cross-entropy-with-variance loss.

loss = mean_{s,b,h,w}[ logsumexp_c(mean + std*eps[s]) - logit[label] ]

Device computes the big term sum_{s,b,pix} ln(sum_c exp(mean + std*eps));
the label term (a 19x-smaller gather, sum_s logit[label]) is folded in on
the host alongside the other input prep (std=exp(0.5*log_var), bf16 casts).

Per-core layout: the core's pixel slab [S,B,C,32768] is viewed as a flat
row axis [S*1216, 2048] (row = (s,b,c,chunk), 2048 pixels each). Each
1216-row s-block is padded to 1280 = 10x128 rows and split into two
[128, 10240] supertiles (SBUF partition p of supertile k holds flat rows
{640k+128j+p}). The host pre-arranges eps/sd/mean in exactly this
partition-major layout, so one supertile DMA is 128 contiguous 20KB
lines (128 descriptors) - a single SWDGE (gpsimd) queue sustains the
pipeline cadence without a second DMA stream contending with the DVE
for SBUF ports (a second stream measurably slows every DVE op ~20%).

Per supertile:
- DVE: t = eps*sd, x = t + mean as two [128,10240] bf16 tensor_tensor
  ops (2x mode), ping-ponging between the eps and t buffers; eps pool
  is quad-buffered so DMAs stay ahead
- ACT: e1 = exp(x) in one op
- PE:  class-sums via 0/1 selector matmuls, 4 x 512-col per tile, into a
  [65, 2048] f32 PSUM block per s-block (psum row = b*16+chunk, row 64 =
  trash for the 64 pad rows); accumulation start/stop spans the 10 tiles
- ACT: ln on the PSUM block in place with accum_out -> [65,1]; the ln
  for s-block s is issued after the first exp of s-block s+1 so it never
  stalls on the matmul tail. Compile is steered to the combined
  natural_log_exp_and_others ACT table set (one table load total).

The first and last supertiles run at single-tile granularity (5 x
[128,2048] units) so the pipeline ramps up in ~10us and drains in ~10us
instead of waiting on whole-supertile dependencies.
"""

import numpy as np
import ml_dtypes

import concourse.bass as bass
import concourse.bacc as bacc
import concourse.mybir as mybir
from concourse import tile
from concourse.bass_interp import get_hw_module
from concourse.bass_utils import run_bass_kernel_spmd
from concourse.mybir import ActivationFunctionType as Act

# ---------------------------------------------------------------- sizes
S, B, C, H, W = 10, 4, 19, 512, 512
HW = H * W
NCORES = 8
SLAB = HW // NCORES          # pixels per (core, b) = 32768
F = 2048                     # pixels per chunk row
CH = SLAB // F               # 16 chunks per (b, c)
QB = B * C * CH              # 1216 real rows per s-block
QP = 1280                    # padded rows per s-block (10 x 128)
NT = QP // 128               # 10 tiles per s-block
ST = 5                       # tiles per supertile
STF = ST * F                 # 10240 free elems per supertile
NK = 2 * S                   # 20 supertiles
NROW = 65                    # psum rows: 64 real (b,chunk) + 1 trash
F32 = mybir.dt.float32
BF16 = mybir.dt.bfloat16


def _compile_combined_act_set(nc):
    """Steer insert_act_table_loads to the one table set that holds both
    Exp and Ln, so the kernel issues a single ACT_TABLE_LOAD instead of
    toggling exp<->ln sets every s-block."""
    orig = bacc.get_activation_tables
    combined = "natural_log_exp_and_others"

    def patched(arch):
        out = {}
        for name, fns in orig(arch).items():
            if name != combined:
                fns = fns - {Act.Exp, Act.Ln}
            out[name] = fns
        return out

    bacc.get_activation_tables = patched
    try:
        nc.compile()
    finally:
        bacc.get_activation_tables = orig


def build_program():
    nc = bacc.Bacc("TRN2", target_bir_lowering=False, debug=False,
                   num_devices=NCORES)

    # partition-major: row k*128+p holds the 10240 free elems of
    # supertile k's partition p
    eps_h = nc.dram_tensor("eps_s", [NK * 128, STF], BF16,
                           kind="ExternalInput")
    sd_h = nc.dram_tensor("sd_s", [2 * 128, STF], BF16,
                          kind="ExternalInput")
    mean_h = nc.dram_tensor("mean_s", [2 * 128, STF], BF16,
                            kind="ExternalInput")
    sel_h = nc.dram_tensor("sel_s", [128, NT * NROW], BF16,
                           kind="ExternalInput")
    lse_h = nc.dram_tensor("lse_out", [64, 1], F32, kind="ExternalOutput")

    def sup_ap(handle, k, j=0, w=ST):
        """[128, w*F] partition-major view of supertile k, tiles j..j+w."""
        return bass.AP(tensor=handle, offset=(k * 128 * ST + j) * F,
                       ap=[[STF, 128], [1, w * F]])

    with tile.TileContext(nc) as tc:
        with (
            tc.tile_pool(name="consts", bufs=1) as consts,
            tc.tile_pool(name="epsp", bufs=4) as eps_pool,
            tc.tile_pool(name="xp", bufs=2) as x_pool,
            tc.tile_pool(name="post", bufs=2) as post,
            tc.tile_pool(name="accp", bufs=1) as acc_pool,
            tc.tile_pool(name="psum", bufs=2, space="PSUM") as psum_pool,
        ):
            sel_t = consts.tile([128, NT * NROW], BF16, tag="sel")
            nc.sync.dma_start(out=sel_t, in_=sel_h.ap())
            # tile-by-tile so mul(tile 0) starts ~10us in
            sd_sb = [consts.tile([128, STF], BF16, tag=f"sd{h}",
                                 name=f"sd{h}") for h in range(2)]
            mean_sb = [consts.tile([128, STF], BF16, tag=f"mean{h}",
                                   name=f"mean{h}") for h in range(2)]
            for hh in range(2):
                for j in range(ST):
                    fsl = slice(j * F, (j + 1) * F)
                    nc.sync.dma_start(out=sd_sb[hh][:, fsl],
                                      in_=sup_ap(sd_h, hh, j, 1))
                    nc.scalar.dma_start(out=mean_sb[hh][:, fsl],
                                        in_=sup_ap(mean_h, hh, j, 1))

            acc = acc_pool.tile([64, 1], F32)
            nc.vector.memset(acc, 0.0)

            def emit_ln(psum_t):
                lse_p = post.tile([NROW, 1], F32, tag="lsep", name="lse_p")
                nc.scalar.activation(psum_t, psum_t, Act.Ln,
                                     accum_out=lse_p)
                nc.vector.tensor_add(acc, acc, lse_p[0:64])

            # supertiles 0 and 2 (h=0 of blocks 0/1) only need the first
            # sd/mean half, so running k = 0,2,1,3 first hides the load
            # of the second half behind real work
            order = [0, 2, 1, 3] + list(range(4, NK))
            psum_tiles = {}
            block_done = {}       # s -> position of its second supertile
            pending = []          # blocks awaiting their deferred ln
            for pos, k in enumerate(order):
                s, h = divmod(k, 2)
                if h == 0:
                    psum_tiles[s] = psum_pool.tile([NROW, 4 * 512], F32,
                                                   tag="ps", name="ps")
                psum_t = psum_tiles[s]
                eps_t = eps_pool.tile([128, STF], BF16, tag="eps")
                if k == 0:
                    units = [(0, 2 * F), (2 * F, 3 * F)]
                elif k == NK - 1:
                    units = [(j * F, F) for j in range(ST)]
                else:
                    units = [(0, STF)]
                if k == 0:
                    for off, w in units:
                        nc.gpsimd.dma_start(
                            out=eps_t[:, off:off + w],
                            in_=sup_ap(eps_h, 0, off // F, w // F))
                else:
                    nc.gpsimd.dma_start(out=eps_t, in_=sup_ap(eps_h, k))
                x_t = x_pool.tile([128, STF], BF16, tag="x")
                for off, w in units:
                    usl = slice(off, off + w)
                    nc.vector.tensor_mul(eps_t[:, usl], eps_t[:, usl],
                                         sd_sb[h][:, usl])
                    nc.vector.tensor_add(x_t[:, usl], eps_t[:, usl],
                                         mean_sb[h][:, usl])
                    nc.scalar.activation(eps_t[:, usl], x_t[:, usl],
                                         Act.Exp)
                    for j in range(off // F, (off + w) // F):
                        tt = ST * h + j
                        ssl = slice(tt * NROW, (tt + 1) * NROW)
                        for m in range(4):
                            msl = slice(j * F + m * 512,
                                        j * F + (m + 1) * 512)
                            nc.tensor.matmul(
                                psum_t[:, m * 512:(m + 1) * 512],
                                sel_t[:, ssl], eps_t[:, msl],
                                start=(tt == 0), stop=(tt == NT - 1))
                if h == 1:
                    block_done[s] = pos
                    pending.append(s)
                # deferred ln: a block's matmul tail completed during a
                # later supertile's exp, so the ln never stalls the ACT
                while pending and block_done[pending[0]] < pos:
                    emit_ln(psum_tiles.pop(pending.pop(0)))

            emit_ln(psum_tiles.pop(pending.pop(0)))
            nc.sync.dma_start(out=lse_h.ap(), in_=acc)

    _compile_combined_act_set(nc)
    nc.m = get_hw_module(nc.m)
    return nc


def _selectors():
    """[128, NT*NROW] 0/1 routing: tile tt row i -> psum row b*16+chunk."""
    bf = ml_dtypes.bfloat16
    sel = np.zeros((128, NT, NROW), dtype=bf)
    for tt in range(NT):
        for i in range(128):
            q = 128 * tt + i
            if q < QB:
                b, rem = divmod(q, C * CH)
                c, chunk = divmod(rem, CH)
                sel[i, tt, b * CH + chunk] = 1.0
            else:
                sel[i, tt, 64] = 1.0
    return sel.reshape(128, NT * NROW)


def _to_partition_major(rows, nblocks):
    """[nblocks*1216, 2048] real rows -> [nblocks*2*128, 10240] where row
    k*128+p = concat_j rows_padded[block, 640*(k%2)+128*j+p]."""
    nb = nblocks
    padded = np.zeros((nb, QP, F), dtype=rows.dtype)
    padded[:, :QB] = rows.reshape(nb, QB, F)
    # [nb, 2, 5, 128, F] -> [nb, 2, 128, 5, F]
    out = padded.reshape(nb, 2, ST, 128, F).swapaxes(2, 3)
    return np.ascontiguousarray(out).reshape(nb * 2 * 128, ST * F)


def kernel(mean, log_var, label, eps, _trace=False):
    bf = ml_dtypes.bfloat16
    mean_r = np.asarray(mean, dtype=np.float32).reshape(B, C, HW)
    log_var_r = np.asarray(log_var, dtype=np.float32).reshape(B, C, HW)
    lab = np.asarray(label).reshape(B, HW)
    eps_r = np.asarray(eps, dtype=np.float32).reshape(S, B, C, HW)

    std = np.exp(0.5 * log_var_r)
    sd_bf = std.astype(bf)
    mean_bf = mean_r.astype(bf)
    eps_bf = eps_r.astype(bf)

    sel = _selectors()
    in_maps = []
    for cid in range(NCORES):
        lo, hi = cid * SLAB, (cid + 1) * SLAB
        in_maps.append({
            "eps_s": _to_partition_major(
                np.ascontiguousarray(eps_bf[:, :, :, lo:hi]), S),
            "sd_s": _to_partition_major(
                np.ascontiguousarray(sd_bf[:, :, lo:hi]), 1),
            "mean_s": _to_partition_major(
                np.ascontiguousarray(mean_bf[:, :, lo:hi]), 1),
            "sel_s": sel,
        })

    nc = build_program()
    res = run_bass_kernel_spmd(
        nc, in_maps, core_ids=list(range(NCORES)), trace=_trace
    )
    global last_results
    last_results = res

    lse_total = np.float64(0.0)
    for cid in range(NCORES):
        lse_total += res.results[cid]["lse_out"].astype(np.float64).sum()

    # label term on host: sum_{s,b,pix} (mean[lab] + std[lab]*eps[s,lab])
    meanl = np.take_along_axis(mean_r, lab[:, None, :], axis=1)[:, 0, :]
    stdl = np.take_along_axis(std, lab[:, None, :], axis=1)[:, 0, :]
    epsl = np.take_along_axis(
        eps_r, lab[None, :, None, :], axis=2)[:, :, 0, :]
    label_total = (S * meanl.astype(np.float64).sum()
                   + (stdl * epsl.sum(axis=0)).astype(np.float64).sum())

    loss = (lse_total - label_total) / float(S * B * HW)
    return np.float32(loss)
